# revision 29
# baseline (speedup 1.0000x reference)
"""MoE fusion kernel for Trainium2, data-parallel across 8 NeuronCores.

Reference computation (per row b of B=16384):
    x      = concat(z_s, z_e)                    # [1024]
    wgt    = softmax(x @ rw + rb)                # [8]
    h_e    = gelu(x @ w1[e] + b1[e])             # [8, 1024]
    y_e    = h_e @ w2[e] + b2[e]                 # [8, 1024]
    ln_e   = (y_e - mu_e) * rsqrt(var_e + eps) * gamma[e] + beta[e]
    z      = sum_e wgt[e] * ln_e                 # [1024]

Sharding: batch split 8 ways (2048 rows/core), params replicated. No
collectives.

GEMMs run in fp8(e4m3) with the tensor engine's DoubleRow perf mode
(contracts 2x128 per matmul at 0.5 cycles/row = 4x the fp32r MAC rate).
To stay within the error budget each GEMM uses three DoubleRow passes
(residual "virtual bf16" quantization):

    a @ w  ~=  a_hi @ w_hi  +  a_lo @ w_hi  +  a_hi @ w_lo

where a_hi = fp8(a), a_lo = fp8(a - a_hi) (residuals stay unscaled:
they fit fp8's dynamic range), and weights are pre-scaled by 128 on the
host so their values sit in fp8's normal range. All passes accumulate
into one fp32 PSUM group whose overall scale is 128; the gelu drain
rescales by 1/128 (activation scale), and the layer-2 LayerNorm is
computed directly on the x128 PSUM (LN is scale-invariant; eps is
scaled by 128^2). b2 rides the layer-2 PSUM group as a K=1 DoubleRow
ones-row matmul of its own fp8 hi/lo pair; b1 rides the gelu
activation's per-partition bias.

Weight residual pairs are quantized on the host (static parameter
preprocessing); activation hi/lo tensors are produced on device (act
casts x_hi from transpose PSUM; DVE casts h_hi and computes the lo
residuals with fused scalar_tensor_tensor). The softmax router runs
the same fp8 hi/lo scheme (logits scale 1/128 folded into the Exp
activation). LayerNorm's rstd is a 3-step Newton rsqrt on DVE
(constant seed) so the act engine never leaves the gelu table set
(act-function table reloads cost ~5us each); the weighted expert
accumulation of normalized outputs runs on the gpsimd (Pool) engine.
The schedule software-pipelines layer 2 one half-tile behind layer 1
through a shared 8-bank PSUM ring.
"""
import numpy as np
import ml_dtypes
from contextlib import ExitStack

import concourse.bass as bass
import concourse.bacc as bacc
import concourse.mybir as mybir
import concourse.tile as tile
from concourse.bass_utils import run_bass_kernel_spmd

P = 128          # partitions
D = 1024         # IN_DIM == OUT_DIM
E = 8            # experts
NK = D // P      # 8 contraction chunks of 128
NC2 = NK // 2    # 4 DoubleRow chunks of 256
NCORES = 8
B_FULL = 16384
BL = B_FULL // NCORES   # 2048 rows per core
SEQ = 512               # z_s/z_e width
WS = 128.0              # host weight pre-scale (power of two)

F32 = mybir.dt.float32
F32R = mybir.dt.float32r
F8 = mybir.dt.float8e4
BF16 = mybir.dt.bfloat16
NP8 = ml_dtypes.float8_e4m3
AF = mybir.ActivationFunctionType
ALU = mybir.AluOpType
DR = mybir.MatmulPerfMode.DoubleRow


def _build(bl, fast_affine, cfg=None):
    """Build the per-core Bass program. bl: rows per core."""
    cfg = cfg or {}
    nb = bl // P            # 128-row chunks
    nt = bl // 512          # 512-row tiles

    nc = bacc.Bacc(None, target_bir_lowering=False)
    zs_d = nc.declare_dram_parameter("zs", [bl, SEQ], F32R, isOutput=False)
    ze_d = nc.declare_dram_parameter("ze", [bl, SEQ], F32R, isOutput=False)
    rwh_d = nc.declare_dram_parameter("rwh", [P, NK, E], F8, isOutput=False)
    rwl_d = nc.declare_dram_parameter("rwl", [P, NK, E], F8, isOutput=False)
    rb_d = nc.declare_dram_parameter("rb", [1, E], F32R, isOutput=False)
    w1h_d = nc.declare_dram_parameter("w1h", [E, P, NK, D], F8, isOutput=False)
    w1l_d = nc.declare_dram_parameter("w1l", [E, P, NK, D], F8, isOutput=False)
    w2h_d = nc.declare_dram_parameter("w2h", [E, P, NK, D], F8, isOutput=False)
    w2l_d = nc.declare_dram_parameter("w2l", [E, P, NK, D], F8, isOutput=False)
    b1_d = nc.declare_dram_parameter("b1", [E, NK, P], F32, isOutput=False)
    b2p_d = nc.declare_dram_parameter("b2p", [E, 1, 2, D], F8, isOutput=False)
    gam_d = nc.declare_dram_parameter("gam", [E, D], F32, isOutput=False)
    bet_d = nc.declare_dram_parameter("bet", [E, D], F32, isOutput=False)
    id_d = nc.declare_dram_parameter("ident", [P, P], F32R, isOutput=False)
    on_d = nc.declare_dram_parameter("ones", [1, P], F32R, isOutput=False)
    on8_d = nc.declare_dram_parameter("ones8", [1, 2, P], F8, isOutput=False)
    z_d = nc.declare_dram_parameter("z", [bl, D], F32, isOutput=True)

    with tile.TileContext(nc) as tc, ExitStack() as ctx:
        consts = ctx.enter_context(tc.tile_pool(name="consts", bufs=1))
        xload = ctx.enter_context(tc.tile_pool(name="xload", bufs=cfg.get("xload", 3)))
        xtp = ctx.enter_context(tc.tile_pool(name="xtp", bufs=1))
        wp = ctx.enter_context(tc.tile_pool(name="wp", bufs=cfg.get("w", 2)))
        bp = ctx.enter_context(tc.tile_pool(name="bp", bufs=cfg.get("bp", 2)))
        hp = ctx.enter_context(tc.tile_pool(name="hp", bufs=cfg.get("h", 2)))
        h32p = ctx.enter_context(tc.tile_pool(name="h32p", bufs=cfg.get("h32", 3)))
        zp = ctx.enter_context(tc.tile_pool(name="zp", bufs=nb))
        cp = ctx.enter_context(tc.tile_pool(name="cp", bufs=cfg.get("cp", 3)))
        sp = ctx.enter_context(tc.tile_pool(name="sp", bufs=8))
        gp = None
        if not fast_affine:
            gp = ctx.enter_context(tc.tile_pool(name="gp", bufs=cfg.get("gp", 1)))
        psA = ctx.enter_context(tc.tile_pool(name="psA", bufs=cfg.get("psA", 8), space="PSUM"))
        psB = psA

        ident = consts.tile([P, P], F32R)
        nc.sync.dma_start(out=ident, in_=id_d[:])
        seed_t = consts.tile([P, 2], F32)
        nc.vector.memset(seed_t, 0.030)
        ones_t = consts.tile([1, P], F32R)
        nc.sync.dma_start(out=ones_t, in_=on_d[:])
        ones8_t = consts.tile([1, 2, P], F8)
        nc.sync.dma_start(out=ones8_t, in_=on8_d[:])
        rwh_sb = consts.tile([P, NK, E], F8)
        nc.sync.dma_start(out=rwh_sb, in_=rwh_d[:])
        rwl_sb = consts.tile([P, NK, E], F8)
        nc.sync.dma_start(out=rwl_sb, in_=rwl_d[:])
        rb_sb = consts.tile([1, E], F32R)
        nc.sync.dma_start(out=rb_sb, in_=rb_d[:])


        def load_expert(e):
            w1h = wp.tile([P, NK, D], F8, tag="w1h", name=f"w1h_{e}")
            nc.sync.dma_start(out=w1h, in_=w1h_d[e])
            w1l = wp.tile([P, NK, D], F8, tag="w1l", name=f"w1l_{e}")
            nc.sync.dma_start(out=w1l, in_=w1l_d[e])
            w2h = wp.tile([P, NK, D], F8, tag="w2h", name=f"w2h_{e}")
            nc.sync.dma_start(out=w2h, in_=w2h_d[e])
            w2l = wp.tile([P, NK, D], F8, tag="w2l", name=f"w2l_{e}")
            nc.sync.dma_start(out=w2l, in_=w2l_d[e])
            b1_sb = bp.tile([P, NK], F32, tag="b1", name=f"b1_{e}")
            nc.sync.dma_start(out=b1_sb, in_=b1_d[e].rearrange("m p -> p m"))
            b2p_sb = bp.tile([1, 2, D], F8, tag="b2", name=f"b2_{e}")
            nc.sync.dma_start(out=b2p_sb, in_=b2p_d[e])
            gam_sb = bet_sb = None
            if not fast_affine:
                gam_sb = gp.tile([P, D], F32, tag="g", name=f"g_{e}")
                nc.sync.dma_start(out=gam_sb, in_=gam_d[e].partition_broadcast(P))
                bet_sb = gp.tile([P, D], F32, tag="bt", name=f"bt_{e}")
                nc.sync.dma_start(out=bet_sb, in_=bet_d[e].partition_broadcast(P))
            return dict(w1h=w1h, w1l=w1l, w2h=w2h, w2l=w2l, b1=b1_sb,
                        b2p=b2p_sb, gam=gam_sb, bet=bet_sb)

        def l1_half(e, t, w, hh, hl, m0, m1):
            """Layer 1: hT chunks [feat 128, batch 512], 3-pass fp8.
            The fp8 hi/lo casts are deferred (returned) so the L2 pair's
            LN chain gets DVE queue priority."""
            deferred = []
            ts = slice(t * 512, (t + 1) * 512)
            w1h, w1l, b1_sb = w["w1h"], w["w1l"], w["b1"]
            for m in range(m0, m1):
                ps_h = psA.tile([P, 512], F32, tag="a", name=f"ph_{e}_{t}_{m}")
                ms = slice(m * P, (m + 1) * P)
                for c in range(NC2):
                    cs = slice(2 * c, 2 * c + 2)
                    nc.tensor.matmul(ps_h, w1h[:, cs, ms], xh[:, cs, ts],
                                     start=(c == 0), stop=False, perf_mode=DR)
                    nc.tensor.matmul(ps_h, w1h[:, cs, ms], xl[:, cs, ts],
                                     start=False, stop=False, perf_mode=DR)
                for c in range(NC2):
                    cs = slice(2 * c, 2 * c + 2)
                    nc.tensor.matmul(ps_h, w1l[:, cs, ms], xh[:, cs, ts],
                                     start=False, stop=(c == NC2 - 1),
                                     perf_mode=DR)
                h32 = h32p.tile([P, 512], BF16, tag="h32", name=f"h32_{e}_{t}_{m}")
                nc.scalar.activation(out=h32, in_=ps_h, func=AF.Gelu,
                                     bias=b1_sb[:, m:m + 1], scale=1.0 / WS)
                deferred.append((h32, m))
            return deferred

        def flush_casts(hh, hl, deferred):
            for h32, m in deferred:
                nc.vector.tensor_scalar(out=hh[:, m, :], in0=h32, scalar1=1.0,
                                        scalar2=None, op0=ALU.mult)
                nc.vector.scalar_tensor_tensor(
                    out=hl[:, m, :], in0=h32, scalar=1.0, in1=hh[:, m, :],
                    op0=ALU.mult, op1=ALU.subtract)

        def l2_pair(e, t, hh, hl, w, sp2):
            """Layer 2 + LN + weighted accumulate, paired row chunks.

            Two 128-row chunks' PSUMs (4 banks) stay live while one packed
            [P, 2] LN-scalar chain (bn_aggr + Newton rsqrt on DVE; no
            act-engine table switch) computes alpha/nbias for both, then
            the act engine applies and Pool accumulates z."""
            w2h, w2l, b2p_sb = w["w2h"], w["w2l"], w["b2p"]
            gam_sb, bet_sb = w["gam"], w["bet"]
            if True:
                ps_pair = []
                stats = sp.tile([P, 2, 2, 6], F32, tag="st",
                                name=f"st_{e}_{t}_{sp2}")
                for sh in range(2):
                    s = sp2 * 2 + sh
                    bb = t * 4 + s
                    ss = slice(s * P, (s + 1) * P)
                    ps_ys = [psB.tile([P, 512], F32, tag="a",
                                      name=f"py_{e}_{bb}_{n}")
                             for n in range(2)]
                    # c-outer: consecutive matmuls share the stationary
                    # h chunk
                    for c in range(NC2):
                        cs = slice(2 * c, 2 * c + 2)
                        for n in range(2):
                            ns = slice(n * 512, (n + 1) * 512)
                            nc.tensor.matmul(ps_ys[n], hh[:, cs, ss],
                                             w2h[:, cs, ns], start=(c == 0),
                                             stop=False, perf_mode=DR)
                            nc.tensor.matmul(ps_ys[n], hh[:, cs, ss],
                                             w2l[:, cs, ns], start=False,
                                             stop=False, perf_mode=DR)
                    for c in range(NC2):
                        cs = slice(2 * c, 2 * c + 2)
                        for n in range(2):
                            nc.tensor.matmul(ps_ys[n], hl[:, cs, ss],
                                             w2h[:, cs, n * 512:(n + 1) * 512],
                                             start=False, stop=False,
                                             perf_mode=DR)
                    for n in range(2):
                        nc.tensor.matmul(ps_ys[n], ones8_t,
                                         b2p_sb[:, :, n * 512:(n + 1) * 512],
                                         start=False, stop=True, perf_mode=DR)
                        nc.vector.bn_stats(out=stats[:, sh, n, :],
                                           in_=ps_ys[n])
                    ps_pair.append(ps_ys)
                mv = sp.tile([P, 2, 2], F32, tag="mv", name=f"mv_{e}_{t}_{sp2}")
                for sh in range(2):
                    nc.vector.bn_aggr(out=mv[:, sh, :], in_=stats[:, sh, :, :])
                vpe = sp.tile([P, 2], F32, tag="vp", name=f"vp_{e}_{t}_{sp2}")
                nc.vector.tensor_scalar(out=vpe, in0=mv[:, :, 1],
                                        scalar1=1e-5 * WS * WS, scalar2=None,
                                        op0=ALU.add)
                # rstd = rsqrt(vpe), Newton from constant seed (typical row
                # std of the x128 psum is ~33 -> rstd ~0.03)
                r = seed_t
                for it in range(3):
                    sq = sp.tile([P, 2], F32, tag=f"sq{it}",
                                 name=f"sq_{e}_{t}_{sp2}_{it}")
                    nc.vector.tensor_mul(sq, r, r)
                    nc.vector.tensor_mul(sq, sq, vpe)
                    nc.vector.tensor_scalar(out=sq, in0=sq, scalar1=-0.5,
                                            scalar2=1.5, op0=ALU.mult,
                                            op1=ALU.add)
                    r2 = sp.tile([P, 2], F32, tag=f"r{it}",
                                 name=f"r_{e}_{t}_{sp2}_{it}")
                    nc.vector.tensor_mul(r2, r, sq)
                    r = r2
                alpha = sp.tile([P, 2], F32, tag="al", name=f"al_{e}_{t}_{sp2}")
                nc.vector.tensor_mul(alpha, r,
                                     wsm[:, t * 4 + sp2 * 2:t * 4 + sp2 * 2 + 2, e])
                nbias = sp.tile([P, 2], F32, tag="nb", name=f"nb_{e}_{t}_{sp2}")
                nc.vector.scalar_tensor_tensor(out=nbias, in0=mv[:, :, 0],
                                               scalar=-1.0, in1=alpha,
                                               op0=ALU.mult, op1=ALU.mult)
                for sh in range(2):
                    s = sp2 * 2 + sh
                    bb = t * 4 + s
                    ps_ys = ps_pair[sh]
                    al_s = alpha[:, sh:sh + 1]
                    nb_s = nbias[:, sh:sh + 1]
                    for n in range(2):
                        ns = slice(n * 512, (n + 1) * 512)
                        zslice = z_t[bb][:, ns]
                        if fast_affine and e == 0:
                            nc.scalar.activation(out=zslice, in_=ps_ys[n],
                                                 func=AF.Identity,
                                                 bias=nb_s, scale=al_s)
                        else:
                            ct = cp.tile([P, 512], F32, tag="ct",
                                         name=f"ct_{e}_{bb}_{n}")
                            nc.scalar.activation(out=ct, in_=ps_ys[n],
                                                 func=AF.Identity,
                                                 bias=nb_s, scale=al_s)
                            if not fast_affine:
                                nc.vector.tensor_mul(ct, ct, gam_sb[:, ns])
                                bw = cp.tile([P, 512], F32, tag="bw",
                                             name=f"bw_{e}_{bb}_{n}")
                                nc.vector.tensor_scalar_mul(
                                    out=bw, in0=bet_sb[:, ns],
                                    scalar1=wsm[:, bb, e:e + 1])
                                if e == 0:
                                    nc.gpsimd.tensor_add(zslice, ct, bw)
                                else:
                                    nc.gpsimd.tensor_add(zslice, zslice, bw)
                                    nc.gpsimd.tensor_add(zslice, zslice, ct)
                            else:
                                nc.gpsimd.tensor_add(zslice, zslice, ct)
                        if e == E - 1 and n == 1:
                            nc.sync.dma_start(
                                out=z_d[bb * P:(bb + 1) * P, :], in_=z_t[bb])



        # ---- transpose x into feature-major fp8 hi/lo + router weights ----
        xh = xtp.tile([P, NK, bl], F8, name="xh")
        xl = xtp.tile([P, NK, bl], F8, name="xl")
        wsm = xtp.tile([P, nb, E], F32, name="wsm")
        for b in range(nb):
            x_sb = xload.tile([P, D], F32R, tag="x", name=f"x_{b}")
            row = b * P
            nc.sync.dma_start(out=x_sb[:, :SEQ], in_=zs_d[row:row + P, :])
            nc.sync.dma_start(out=x_sb[:, SEQ:], in_=ze_d[row:row + P, :])
            for q in range(2):
                tp = psA.tile([P, 4, P], F32R, tag="a", name=f"tp_{b}_{q}")
                for j in range(4):
                    c = q * 4 + j
                    nc.tensor.transpose(tp[:, j, :], x_sb[:, c * P:(c + 1) * P],
                                        ident)
                nc.scalar.activation(
                    out=xh[:, q * 4:(q + 1) * 4, b * P:(b + 1) * P], in_=tp,
                    func=AF.Copy)
                nc.vector.scalar_tensor_tensor(
                    out=xl[:, q * 4:(q + 1) * 4, b * P:(b + 1) * P], in0=tp,
                    scalar=1.0, in1=xh[:, q * 4:(q + 1) * 4, b * P:(b + 1) * P],
                    op0=ALU.mult, op1=ALU.subtract)
            # router logits for this 128-row chunk (fp8 hi/lo + f32r bias row)
            ps_r = psA.tile([P, E], F32, tag="a", name=f"psr_{b}")
            for c in range(NC2):
                nc.tensor.matmul(ps_r, xh[:, 2 * c:2 * c + 2, b * P:(b + 1) * P],
                                 rwh_sb[:, 2 * c:2 * c + 2, :],
                                 start=(c == 0), stop=False, perf_mode=DR)
            for c in range(NC2):
                nc.tensor.matmul(ps_r, xh[:, 2 * c:2 * c + 2, b * P:(b + 1) * P],
                                 rwl_sb[:, 2 * c:2 * c + 2, :],
                                 start=False, stop=False, perf_mode=DR)
                nc.tensor.matmul(ps_r, xl[:, 2 * c:2 * c + 2, b * P:(b + 1) * P],
                                 rwh_sb[:, 2 * c:2 * c + 2, :],
                                 start=False, stop=False, perf_mode=DR)
            nc.tensor.matmul(ps_r, ones_t, rb_sb, start=False, stop=True)
            ex = sp.tile([P, E], F32, tag="ex", name=f"ex_{b}")
            nc.scalar.activation(out=ex, in_=ps_r, func=AF.Exp, scale=1.0 / WS)
            sm = sp.tile([P, 1], F32, tag="sm", name=f"sm_{b}")
            nc.vector.tensor_reduce(out=sm, in_=ex, axis=mybir.AxisListType.X,
                                    op=ALU.add)
            rc = sp.tile([P, 1], F32, tag="rc", name=f"rc_{b}")
            nc.vector.reciprocal(out=rc, in_=sm)
            nc.vector.tensor_scalar_mul(out=wsm[:, b, :], in0=ex, scalar1=rc)

        z_t = [zp.tile([P, D], F32, tag="z", name=f"z_{b}") for b in range(nb)]

        # ---- pipelined expert loop: L2 lags L1 by one 512-row tile ----
        # (expert 0's weights were DMA'd before the x preamble so they are
        # resident when its first L1 matmuls issue)
        pend = None
        wcur = None
        for k in range(E * nt + 1):
            if k < E * nt:
                e, t = divmod(k, nt)
                if t == 0:
                    wcur = load_expert(e)
                hh = hp.tile([P, NK, 512], F8, tag="hh", name=f"hh_{e}_{t}")
                hl = hp.tile([P, NK, 512], F8, tag="hl", name=f"hl_{e}_{t}")
                d0 = l1_half(e, t, wcur, hh, hl, 0, NK // 2)
                if pend is not None:
                    l2_pair(pend[0], pend[1], pend[2], pend[3], pend[4], 0)
                flush_casts(hh, hl, d0)
                d1 = l1_half(e, t, wcur, hh, hl, NK // 2, NK)
                if pend is not None:
                    l2_pair(pend[0], pend[1], pend[2], pend[3], pend[4], 1)
                flush_casts(hh, hl, d1)
                pend = (e, t, hh, hl, wcur)
            else:
                l2_pair(pend[0], pend[1], pend[2], pend[3], pend[4], 0)
                l2_pair(pend[0], pend[1], pend[2], pend[3], pend[4], 1)

    nc.compile()
    return nc


_NC_CACHE = {}
_RUNNER_CACHE = {}


def _pjrt_runner(nc):
    """Reusable jitted PJRT executable for `nc` (axon path). Mirrors
    bass2jax.run_bass_via_pjrt but is cached so repeated kernel() calls do
    not re-trace/recompile."""
    import jax
    from jax.sharding import Mesh, PartitionSpec
    from jax.experimental.shard_map import shard_map
    from concourse.bass2jax import (_bass_exec_p, install_neuronx_cc_hook,
                                    partition_id_tensor)

    install_neuronx_cc_hook()
    partition_name = nc.partition_id_tensor.name if nc.partition_id_tensor else None
    in_names, out_names, out_avals = [], [], []
    for alloc in nc.m.functions[0].allocations:
        if not isinstance(alloc, mybir.MemoryLocationSet):
            continue
        name = alloc.memorylocations[0].name
        if alloc.kind == "ExternalInput":
            if name != partition_name:
                in_names.append(name)
        elif alloc.kind == "ExternalOutput":
            out_names.append(name)
            out_avals.append(jax.core.ShapedArray(tuple(alloc.tensor_shape),
                                                  mybir.dt.np(alloc.dtype)))
    n_params = len(in_names)
    all_in = list(in_names) + list(out_names)
    if partition_name is not None:
        all_in.append(partition_name)

    def _body(*args):
        operands = list(args)
        if partition_name is not None:
            operands.append(partition_id_tensor())
        return tuple(_bass_exec_p.bind(
            *operands, out_avals=tuple(out_avals), in_names=tuple(all_in),
            out_names=tuple(out_names), lowering_input_output_aliases=(),
            sim_require_finite=True, sim_require_nnan=True, nc=nc))

    devices = jax.devices()[:NCORES]
    assert len(devices) == NCORES
    mesh = Mesh(np.asarray(devices), ("core",))
    specs = (PartitionSpec("core"),) * (n_params + len(out_names))
    fn = jax.jit(shard_map(_body, mesh=mesh, in_specs=specs,
                           out_specs=(PartitionSpec("core"),) * len(out_names),
                           check_rep=False), keep_unused=True)
    return fn, in_names, out_names, out_avals


def _run_cached(nc, in_maps):
    """Run via cached jitted executable with retry; fall back to
    run_bass_kernel_spmd. Retries cover transient device wedges
    (NRT_EXEC_UNIT_UNRECOVERABLE) seen after rapid process turnover."""
    import time as _time
    last_exc = None
    for attempt in range(3):
        try:
            return _run_once(nc, in_maps)
        except Exception as e:
            last_exc = e
            _RUNNER_CACHE.pop(id(nc), None)
            _time.sleep(10 * (attempt + 1))
    raise last_exc


def _run_once(nc, in_maps):
    import jax
    try:
        from concourse._compat import axon_active
        if not axon_active():
            raise RuntimeError("not axon; use native path")
        key = id(nc)
        if key not in _RUNNER_CACHE:
            _RUNNER_CACHE[key] = _pjrt_runner(nc)
        fn, in_names, out_names, out_avals = _RUNNER_CACHE[key]
        concat_in = [np.concatenate([np.asarray(in_maps[c][k])
                                     for c in range(NCORES)], axis=0)
                     for k in in_names]
        concat_zeros = [np.zeros((NCORES * a.shape[0], *a.shape[1:]), a.dtype)
                        for a in out_avals]
        outs = fn(*concat_in, *concat_zeros)
        jax.block_until_ready(outs)
        out_np = [np.asarray(o) for o in outs]
        return [{name: out_np[i].reshape(NCORES, *out_avals[i].shape)[c]
                 for i, name in enumerate(out_names)}
                for c in range(NCORES)]
    except Exception:
        res = run_bass_kernel_spmd(nc, in_maps, core_ids=list(range(NCORES)))
        return res.results


def _get_nc(bl, fast_affine):
    key = (bl, fast_affine)
    if key not in _NC_CACHE:
        cfg = None if fast_affine else {"xload": 2, "cp": 1, "gp": 1, "h32": 1, "bp": 1}
        _NC_CACHE[key] = _build(bl, fast_affine, cfg)
    return _NC_CACHE[key]


def _q8_pair(a):
    """fp8 hi/lo residual pair of a (f32). Residual left unscaled (it fits
    fp8's dynamic range); hi + lo together carry ~bf16 precision."""
    hi = a.astype(NP8)
    lo = (a - hi.astype(np.float32)).astype(NP8)
    return hi, lo


def _w_layout(w):
    """[D_in, D_out] -> [P, NK, D_out] with dim1 = (chunk c, plane i):
    input feature f = c*256 + i*128 + p."""
    d_in, d_out = w.shape
    return np.ascontiguousarray(
        w.reshape(d_in // 256, 2, P, d_out).transpose(2, 0, 1, 3)
        .reshape(P, d_in // P, d_out))


def kernel(z_s, z_e, router_w, router_b, w1, b1, w2, b2, gamma, beta):
    z_s = np.ascontiguousarray(np.asarray(z_s, dtype=np.float32))
    z_e = np.ascontiguousarray(np.asarray(z_e, dtype=np.float32))
    router_w = np.asarray(router_w, dtype=np.float32)
    router_b = np.asarray(router_b, dtype=np.float32)
    w1 = np.asarray(w1, dtype=np.float32)
    b1 = np.asarray(b1, dtype=np.float32)
    w2 = np.asarray(w2, dtype=np.float32)
    b2 = np.asarray(b2, dtype=np.float32)
    gamma = np.ascontiguousarray(np.asarray(gamma, dtype=np.float32))
    beta = np.ascontiguousarray(np.asarray(beta, dtype=np.float32))

    b_full = z_s.shape[0]
    assert b_full % NCORES == 0, f"batch {b_full} not divisible by {NCORES} cores"
    bl = b_full // NCORES
    assert bl % 512 == 0, f"per-core batch {bl} must be a multiple of 512"

    fast_affine = bool(np.all(gamma == 1.0) and np.all(beta == 0.0))
    nc = _get_nc(bl, fast_affine)

    # static parameter preprocessing: fp8 residual pairs in PE layout
    rwh, rwl = _q8_pair(WS * router_w)
    rwh = _w_layout(rwh.astype(np.float32)).astype(NP8)
    rwl = _w_layout(rwl.astype(np.float32)).astype(NP8)
    w1h_l, w1l_l, w2h_l, w2l_l = [], [], [], []
    for e in range(E):
        h, l = _q8_pair(WS * w1[e])
        w1h_l.append(_w_layout(h.astype(np.float32)))
        w1l_l.append(_w_layout(l.astype(np.float32)))
        h, l = _q8_pair(WS * w2[e])
        w2h_l.append(_w_layout(h.astype(np.float32)))
        w2l_l.append(_w_layout(l.astype(np.float32)))
    w1h = np.stack(w1h_l).astype(NP8)
    w1l = np.stack(w1l_l).astype(NP8)
    w2h = np.stack(w2h_l).astype(NP8)
    w2l = np.stack(w2l_l).astype(NP8)
    b1_r = np.ascontiguousarray(b1.reshape(E, NK, P))
    b2h, b2l = _q8_pair(WS * b2)
    b2p = np.ascontiguousarray(
        np.stack([b2h, b2l], axis=1).reshape(E, 1, 2, D))
    rb_r = np.ascontiguousarray((WS * router_b).reshape(1, E))

    ident = np.eye(P, dtype=np.float32)
    ones_h = np.ones((1, P), dtype=np.float32)
    ones8 = np.ones((1, 2, P), dtype=np.float32).astype(NP8)
    in_maps = []
    for c in range(NCORES):
        sl = slice(c * bl, (c + 1) * bl)
        in_maps.append({
            "zs": z_s[sl], "ze": z_e[sl],
            "rwh": rwh, "rwl": rwl, "rb": rb_r,
            "w1h": w1h, "w1l": w1l, "w2h": w2h, "w2l": w2l,
            "b1": b1_r, "b2p": b2p,
            "gam": gamma, "bet": beta,
            "ident": ident, "ones": ones_h, "ones8": ones8,
        })
    results = _run_cached(nc, in_maps)
    return np.concatenate([results[c]["z"] for c in range(NCORES)], axis=0)


# revision 31
# speedup vs baseline: 1.0033x; 1.0033x over previous
"""MoE fusion kernel for Trainium2, data-parallel across 8 NeuronCores.

Reference computation (per row b of B=16384):
    x      = concat(z_s, z_e)                    # [1024]
    wgt    = softmax(x @ rw + rb)                # [8]
    h_e    = gelu(x @ w1[e] + b1[e])             # [8, 1024]
    y_e    = h_e @ w2[e] + b2[e]                 # [8, 1024]
    ln_e   = (y_e - mu_e) * rsqrt(var_e + eps) * gamma[e] + beta[e]
    z      = sum_e wgt[e] * ln_e                 # [1024]

Sharding: batch split 8 ways (2048 rows/core), params replicated. No
collectives.

GEMMs run in fp8(e4m3) with the tensor engine's DoubleRow perf mode
(contracts 2x128 per matmul at 0.5 cycles/row = 4x the fp32r MAC rate).
To stay within the error budget each GEMM uses three DoubleRow passes
(residual "virtual bf16" quantization):

    a @ w  ~=  a_hi @ w_hi  +  a_lo @ w_hi  +  a_hi @ w_lo

where a_hi = fp8(a), a_lo = fp8(a - a_hi) (residuals stay unscaled:
they fit fp8's dynamic range), and weights are pre-scaled by 128 on the
host so their values sit in fp8's normal range. All passes accumulate
into one fp32 PSUM group whose overall scale is 128; the gelu drain
rescales by 1/128 (activation scale), and the layer-2 LayerNorm is
computed directly on the x128 PSUM (LN is scale-invariant; eps is
scaled by 128^2). b2 rides the layer-2 PSUM group as a K=1 DoubleRow
ones-row matmul of its own fp8 hi/lo pair; b1 rides the gelu
activation's per-partition bias.

Weight residual pairs are quantized on the host (static parameter
preprocessing); activation hi/lo tensors are produced on device (act
casts x_hi from transpose PSUM; DVE casts h_hi and computes the lo
residuals with fused scalar_tensor_tensor). The softmax router runs
the same fp8 hi/lo scheme (logits scale 1/128 folded into the Exp
activation). LayerNorm's rstd is a 3-step Newton rsqrt on DVE
(constant seed) so the act engine never leaves the gelu table set
(act-function table reloads cost ~5us each); the weighted expert
accumulation of normalized outputs runs on the gpsimd (Pool) engine.
The schedule software-pipelines layer 2 one half-tile behind layer 1
through a shared 8-bank PSUM ring.
"""
import numpy as np
import ml_dtypes
from contextlib import ExitStack

import concourse.bass as bass
import concourse.bacc as bacc
import concourse.mybir as mybir
import concourse.tile as tile
from concourse.bass_utils import run_bass_kernel_spmd

P = 128          # partitions
D = 1024         # IN_DIM == OUT_DIM
E = 8            # experts
NK = D // P      # 8 contraction chunks of 128
NC2 = NK // 2    # 4 DoubleRow chunks of 256
NCORES = 8
B_FULL = 16384
BL = B_FULL // NCORES   # 2048 rows per core
SEQ = 512               # z_s/z_e width
WS = 128.0              # host weight pre-scale (power of two)

F32 = mybir.dt.float32
F32R = mybir.dt.float32r
F8 = mybir.dt.float8e4
BF16 = mybir.dt.bfloat16
NP8 = ml_dtypes.float8_e4m3
AF = mybir.ActivationFunctionType
ALU = mybir.AluOpType
DR = mybir.MatmulPerfMode.DoubleRow


def _build(bl, fast_affine, cfg=None):
    """Build the per-core Bass program. bl: rows per core."""
    cfg = cfg or {}
    nb = bl // P            # 128-row chunks
    nt = bl // 512          # 512-row tiles

    nc = bacc.Bacc(None, target_bir_lowering=False)
    zs_d = nc.declare_dram_parameter("zs", [bl, SEQ], BF16, isOutput=False)
    ze_d = nc.declare_dram_parameter("ze", [bl, SEQ], BF16, isOutput=False)
    rwh_d = nc.declare_dram_parameter("rwh", [P, NK, E], F8, isOutput=False)
    rwl_d = nc.declare_dram_parameter("rwl", [P, NK, E], F8, isOutput=False)
    rb_d = nc.declare_dram_parameter("rb", [1, E], F32R, isOutput=False)
    w1h_d = nc.declare_dram_parameter("w1h", [E, P, NK, D], F8, isOutput=False)
    w1l_d = nc.declare_dram_parameter("w1l", [E, P, NK, D], F8, isOutput=False)
    w2h_d = nc.declare_dram_parameter("w2h", [E, P, NK, D], F8, isOutput=False)
    w2l_d = nc.declare_dram_parameter("w2l", [E, P, NK, D], F8, isOutput=False)
    b1_d = nc.declare_dram_parameter("b1", [E, NK, P], F32, isOutput=False)
    b2p_d = nc.declare_dram_parameter("b2p", [E, 1, 2, D], F8, isOutput=False)
    gam_d = nc.declare_dram_parameter("gam", [E, D], F32, isOutput=False)
    bet_d = nc.declare_dram_parameter("bet", [E, D], F32, isOutput=False)
    id_d = nc.declare_dram_parameter("ident", [P, P], BF16, isOutput=False)
    on_d = nc.declare_dram_parameter("ones", [1, P], F32R, isOutput=False)
    on8_d = nc.declare_dram_parameter("ones8", [1, 2, P], F8, isOutput=False)
    z_d = nc.declare_dram_parameter("z", [bl, D], F32, isOutput=True)

    with tile.TileContext(nc) as tc, ExitStack() as ctx:
        consts = ctx.enter_context(tc.tile_pool(name="consts", bufs=1))
        xload = ctx.enter_context(tc.tile_pool(name="xload", bufs=cfg.get("xload", 4)))
        xtp = ctx.enter_context(tc.tile_pool(name="xtp", bufs=1))
        wp = ctx.enter_context(tc.tile_pool(name="wp", bufs=cfg.get("w", 2)))
        bp = ctx.enter_context(tc.tile_pool(name="bp", bufs=cfg.get("bp", 2)))
        hp = ctx.enter_context(tc.tile_pool(name="hp", bufs=cfg.get("h", 2)))
        h32p = ctx.enter_context(tc.tile_pool(name="h32p", bufs=cfg.get("h32", 3)))
        zp = ctx.enter_context(tc.tile_pool(name="zp", bufs=nb))
        cp = ctx.enter_context(tc.tile_pool(name="cp", bufs=cfg.get("cp", 3)))
        sp = ctx.enter_context(tc.tile_pool(name="sp", bufs=8))
        gp = None
        if not fast_affine:
            gp = ctx.enter_context(tc.tile_pool(name="gp", bufs=cfg.get("gp", 1)))
        psA = ctx.enter_context(tc.tile_pool(name="psA", bufs=cfg.get("psA", 8), space="PSUM"))
        psB = psA

        ident = consts.tile([P, P], BF16)
        nc.sync.dma_start(out=ident, in_=id_d[:])
        seed_t = consts.tile([P, 2], F32)
        nc.vector.memset(seed_t, 0.030)
        ones_t = consts.tile([1, P], F32R)
        nc.sync.dma_start(out=ones_t, in_=on_d[:])
        ones8_t = consts.tile([1, 2, P], F8)
        nc.sync.dma_start(out=ones8_t, in_=on8_d[:])
        rwh_sb = consts.tile([P, NK, E], F8)
        nc.sync.dma_start(out=rwh_sb, in_=rwh_d[:])
        rwl_sb = consts.tile([P, NK, E], F8)
        nc.sync.dma_start(out=rwl_sb, in_=rwl_d[:])
        rb_sb = consts.tile([1, E], F32R)
        nc.sync.dma_start(out=rb_sb, in_=rb_d[:])


        def load_expert(e):
            w1h = wp.tile([P, NK, D], F8, tag="w1h", name=f"w1h_{e}")
            nc.sync.dma_start(out=w1h, in_=w1h_d[e])
            w1l = wp.tile([P, NK, D], F8, tag="w1l", name=f"w1l_{e}")
            nc.sync.dma_start(out=w1l, in_=w1l_d[e])
            w2h = wp.tile([P, NK, D], F8, tag="w2h", name=f"w2h_{e}")
            nc.sync.dma_start(out=w2h, in_=w2h_d[e])
            w2l = wp.tile([P, NK, D], F8, tag="w2l", name=f"w2l_{e}")
            nc.sync.dma_start(out=w2l, in_=w2l_d[e])
            b1_sb = bp.tile([P, NK], F32, tag="b1", name=f"b1_{e}")
            nc.sync.dma_start(out=b1_sb, in_=b1_d[e].rearrange("m p -> p m"))
            b2p_sb = bp.tile([1, 2, D], F8, tag="b2", name=f"b2_{e}")
            nc.sync.dma_start(out=b2p_sb, in_=b2p_d[e])
            gam_sb = bet_sb = None
            if not fast_affine:
                gam_sb = gp.tile([P, D], F32, tag="g", name=f"g_{e}")
                nc.sync.dma_start(out=gam_sb, in_=gam_d[e].partition_broadcast(P))
                bet_sb = gp.tile([P, D], F32, tag="bt", name=f"bt_{e}")
                nc.sync.dma_start(out=bet_sb, in_=bet_d[e].partition_broadcast(P))
            return dict(w1h=w1h, w1l=w1l, w2h=w2h, w2l=w2l, b1=b1_sb,
                        b2p=b2p_sb, gam=gam_sb, bet=bet_sb)

        def l1_half(e, t, w, hh, hl, m0, m1):
            """Layer 1: hT chunks [feat 128, batch 512], 3-pass fp8.
            The fp8 hi/lo casts are deferred (returned) so the L2 pair's
            LN chain gets DVE queue priority."""
            deferred = []
            ts = slice(t * 512, (t + 1) * 512)
            w1h, w1l, b1_sb = w["w1h"], w["w1l"], w["b1"]
            for m in range(m0, m1):
                ps_h = psA.tile([P, 512], F32, tag="a", name=f"ph_{e}_{t}_{m}")
                ms = slice(m * P, (m + 1) * P)
                for c in range(NC2):
                    cs = slice(2 * c, 2 * c + 2)
                    nc.tensor.matmul(ps_h, w1h[:, cs, ms], xh[:, cs, ts],
                                     start=(c == 0), stop=False, perf_mode=DR)
                    nc.tensor.matmul(ps_h, w1h[:, cs, ms], xl[:, cs, ts],
                                     start=False, stop=False, perf_mode=DR)
                for c in range(NC2):
                    cs = slice(2 * c, 2 * c + 2)
                    nc.tensor.matmul(ps_h, w1l[:, cs, ms], xh[:, cs, ts],
                                     start=False, stop=(c == NC2 - 1),
                                     perf_mode=DR)
                h32 = h32p.tile([P, 512], BF16, tag="h32", name=f"h32_{e}_{t}_{m}")
                nc.scalar.activation(out=h32, in_=ps_h, func=AF.Gelu,
                                     bias=b1_sb[:, m:m + 1], scale=1.0 / WS)
                deferred.append((h32, m))
            return deferred

        def flush_casts(hh, hl, deferred):
            for h32, m in deferred:
                nc.vector.tensor_scalar(out=hh[:, m, :], in0=h32, scalar1=1.0,
                                        scalar2=None, op0=ALU.mult)
                nc.vector.scalar_tensor_tensor(
                    out=hl[:, m, :], in0=h32, scalar=1.0, in1=hh[:, m, :],
                    op0=ALU.mult, op1=ALU.subtract)

        def l2_pair(e, t, hh, hl, w, sp2):
            """Layer 2 + LN + weighted accumulate, paired row chunks.

            Two 128-row chunks' PSUMs (4 banks) stay live while one packed
            [P, 2] LN-scalar chain (bn_aggr + Newton rsqrt on DVE; no
            act-engine table switch) computes alpha/nbias for both, then
            the act engine applies and Pool accumulates z."""
            w2h, w2l, b2p_sb = w["w2h"], w["w2l"], w["b2p"]
            gam_sb, bet_sb = w["gam"], w["bet"]
            if True:
                ps_pair = []
                stats = sp.tile([P, 2, 2, 6], F32, tag="st",
                                name=f"st_{e}_{t}_{sp2}")
                for sh in range(2):
                    s = sp2 * 2 + sh
                    bb = t * 4 + s
                    ss = slice(s * P, (s + 1) * P)
                    ps_ys = [psB.tile([P, 512], F32, tag="a",
                                      name=f"py_{e}_{bb}_{n}")
                             for n in range(2)]
                    # c-outer: consecutive matmuls share the stationary
                    # h chunk
                    for c in range(NC2):
                        cs = slice(2 * c, 2 * c + 2)
                        for n in range(2):
                            ns = slice(n * 512, (n + 1) * 512)
                            nc.tensor.matmul(ps_ys[n], hh[:, cs, ss],
                                             w2h[:, cs, ns], start=(c == 0),
                                             stop=False, perf_mode=DR)
                            nc.tensor.matmul(ps_ys[n], hh[:, cs, ss],
                                             w2l[:, cs, ns], start=False,
                                             stop=False, perf_mode=DR)
                    for c in range(NC2):
                        cs = slice(2 * c, 2 * c + 2)
                        for n in range(2):
                            nc.tensor.matmul(ps_ys[n], hl[:, cs, ss],
                                             w2h[:, cs, n * 512:(n + 1) * 512],
                                             start=False, stop=False,
                                             perf_mode=DR)
                    for n in range(2):
                        nc.tensor.matmul(ps_ys[n], ones8_t,
                                         b2p_sb[:, :, n * 512:(n + 1) * 512],
                                         start=False, stop=True, perf_mode=DR)
                        nc.vector.bn_stats(out=stats[:, sh, n, :],
                                           in_=ps_ys[n])
                    ps_pair.append(ps_ys)
                mv = sp.tile([P, 2, 2], F32, tag="mv", name=f"mv_{e}_{t}_{sp2}")
                for sh in range(2):
                    nc.vector.bn_aggr(out=mv[:, sh, :], in_=stats[:, sh, :, :])
                vpe = sp.tile([P, 2], F32, tag="vp", name=f"vp_{e}_{t}_{sp2}")
                nc.vector.tensor_scalar(out=vpe, in0=mv[:, :, 1],
                                        scalar1=1e-5 * WS * WS, scalar2=None,
                                        op0=ALU.add)
                # rstd = rsqrt(vpe), Newton from constant seed (typical row
                # std of the x128 psum is ~33 -> rstd ~0.03)
                r = seed_t
                for it in range(3):
                    sq = sp.tile([P, 2], F32, tag=f"sq{it}",
                                 name=f"sq_{e}_{t}_{sp2}_{it}")
                    nc.vector.tensor_mul(sq, r, r)
                    nc.vector.tensor_mul(sq, sq, vpe)
                    nc.vector.tensor_scalar(out=sq, in0=sq, scalar1=-0.5,
                                            scalar2=1.5, op0=ALU.mult,
                                            op1=ALU.add)
                    r2 = sp.tile([P, 2], F32, tag=f"r{it}",
                                 name=f"r_{e}_{t}_{sp2}_{it}")
                    nc.vector.tensor_mul(r2, r, sq)
                    r = r2
                alpha = sp.tile([P, 2], F32, tag="al", name=f"al_{e}_{t}_{sp2}")
                nc.vector.tensor_mul(alpha, r,
                                     wsm[:, t * 4 + sp2 * 2:t * 4 + sp2 * 2 + 2, e])
                nbias = sp.tile([P, 2], F32, tag="nb", name=f"nb_{e}_{t}_{sp2}")
                nc.vector.scalar_tensor_tensor(out=nbias, in0=mv[:, :, 0],
                                               scalar=-1.0, in1=alpha,
                                               op0=ALU.mult, op1=ALU.mult)
                for sh in range(2):
                    s = sp2 * 2 + sh
                    bb = t * 4 + s
                    ps_ys = ps_pair[sh]
                    al_s = alpha[:, sh:sh + 1]
                    nb_s = nbias[:, sh:sh + 1]
                    for n in range(2):
                        ns = slice(n * 512, (n + 1) * 512)
                        zslice = z_t[bb][:, ns]
                        if fast_affine and e == 0:
                            nc.scalar.activation(out=zslice, in_=ps_ys[n],
                                                 func=AF.Identity,
                                                 bias=nb_s, scale=al_s)
                        else:
                            ct = cp.tile([P, 512], F32, tag="ct",
                                         name=f"ct_{e}_{bb}_{n}")
                            nc.scalar.activation(out=ct, in_=ps_ys[n],
                                                 func=AF.Identity,
                                                 bias=nb_s, scale=al_s)
                            if not fast_affine:
                                nc.vector.tensor_mul(ct, ct, gam_sb[:, ns])
                                bw = cp.tile([P, 512], F32, tag="bw",
                                             name=f"bw_{e}_{bb}_{n}")
                                nc.vector.tensor_scalar_mul(
                                    out=bw, in0=bet_sb[:, ns],
                                    scalar1=wsm[:, bb, e:e + 1])
                                if e == 0:
                                    nc.gpsimd.tensor_add(zslice, ct, bw)
                                else:
                                    nc.gpsimd.tensor_add(zslice, zslice, bw)
                                    nc.gpsimd.tensor_add(zslice, zslice, ct)
                            else:
                                nc.gpsimd.tensor_add(zslice, zslice, ct)
                        if e == E - 1 and n == 1:
                            nc.sync.dma_start(
                                out=z_d[bb * P:(bb + 1) * P, :], in_=z_t[bb])



        # ---- transpose x into feature-major fp8 hi/lo + router weights ----
        xh = xtp.tile([P, NK, bl], F8, name="xh")
        xl = xtp.tile([P, NK, bl], F8, name="xl")
        wsm = xtp.tile([P, nb, E], F32, name="wsm")
        for b in range(nb):
            x_sb = xload.tile([P, D], BF16, tag="x", name=f"x_{b}")
            row = b * P
            nc.sync.dma_start(out=x_sb[:, :SEQ], in_=zs_d[row:row + P, :])
            nc.sync.dma_start(out=x_sb[:, SEQ:], in_=ze_d[row:row + P, :])
            for q in range(2):
                tp = psA.tile([P, 4, P], BF16, tag="a", name=f"tp_{b}_{q}")
                for j in range(4):
                    c = q * 4 + j
                    nc.tensor.transpose(tp[:, j, :], x_sb[:, c * P:(c + 1) * P],
                                        ident)
                nc.scalar.activation(
                    out=xh[:, q * 4:(q + 1) * 4, b * P:(b + 1) * P], in_=tp,
                    func=AF.Copy)
                nc.vector.scalar_tensor_tensor(
                    out=xl[:, q * 4:(q + 1) * 4, b * P:(b + 1) * P], in0=tp,
                    scalar=1.0, in1=xh[:, q * 4:(q + 1) * 4, b * P:(b + 1) * P],
                    op0=ALU.mult, op1=ALU.subtract)
            # router logits for this 128-row chunk (fp8 hi/lo + f32r bias row)
            ps_r = psA.tile([P, E], F32, tag="a", name=f"psr_{b}")
            for c in range(NC2):
                nc.tensor.matmul(ps_r, xh[:, 2 * c:2 * c + 2, b * P:(b + 1) * P],
                                 rwh_sb[:, 2 * c:2 * c + 2, :],
                                 start=(c == 0), stop=False, perf_mode=DR)
            for c in range(NC2):
                nc.tensor.matmul(ps_r, xh[:, 2 * c:2 * c + 2, b * P:(b + 1) * P],
                                 rwl_sb[:, 2 * c:2 * c + 2, :],
                                 start=False, stop=False, perf_mode=DR)
                nc.tensor.matmul(ps_r, xl[:, 2 * c:2 * c + 2, b * P:(b + 1) * P],
                                 rwh_sb[:, 2 * c:2 * c + 2, :],
                                 start=False, stop=False, perf_mode=DR)
            nc.tensor.matmul(ps_r, ones_t, rb_sb, start=False, stop=True)
            ex = sp.tile([P, E], F32, tag="ex", name=f"ex_{b}")
            nc.scalar.activation(out=ex, in_=ps_r, func=AF.Exp, scale=1.0 / WS)
            sm = sp.tile([P, 1], F32, tag="sm", name=f"sm_{b}")
            nc.vector.tensor_reduce(out=sm, in_=ex, axis=mybir.AxisListType.X,
                                    op=ALU.add)
            rc = sp.tile([P, 1], F32, tag="rc", name=f"rc_{b}")
            nc.vector.reciprocal(out=rc, in_=sm)
            nc.vector.tensor_scalar_mul(out=wsm[:, b, :], in0=ex, scalar1=rc)

        z_t = [zp.tile([P, D], F32, tag="z", name=f"z_{b}") for b in range(nb)]

        # ---- pipelined expert loop: L2 lags L1 by one 512-row tile ----
        # (expert 0's weights were DMA'd before the x preamble so they are
        # resident when its first L1 matmuls issue)
        pend = None
        wcur = None
        for k in range(E * nt + 1):
            if k < E * nt:
                e, t = divmod(k, nt)
                if t == 0:
                    wcur = load_expert(e)
                hh = hp.tile([P, NK, 512], F8, tag="hh", name=f"hh_{e}_{t}")
                hl = hp.tile([P, NK, 512], F8, tag="hl", name=f"hl_{e}_{t}")
                d0 = l1_half(e, t, wcur, hh, hl, 0, NK // 2)
                if pend is not None:
                    l2_pair(pend[0], pend[1], pend[2], pend[3], pend[4], 0)
                flush_casts(hh, hl, d0)
                d1 = l1_half(e, t, wcur, hh, hl, NK // 2, NK)
                if pend is not None:
                    l2_pair(pend[0], pend[1], pend[2], pend[3], pend[4], 1)
                flush_casts(hh, hl, d1)
                pend = (e, t, hh, hl, wcur)
            else:
                l2_pair(pend[0], pend[1], pend[2], pend[3], pend[4], 0)
                l2_pair(pend[0], pend[1], pend[2], pend[3], pend[4], 1)

    nc.compile()
    return nc


_NC_CACHE = {}
_RUNNER_CACHE = {}


def _pjrt_runner(nc):
    """Reusable jitted PJRT executable for `nc` (axon path). Mirrors
    bass2jax.run_bass_via_pjrt but is cached so repeated kernel() calls do
    not re-trace/recompile."""
    import jax
    from jax.sharding import Mesh, PartitionSpec
    from jax.experimental.shard_map import shard_map
    from concourse.bass2jax import (_bass_exec_p, install_neuronx_cc_hook,
                                    partition_id_tensor)

    install_neuronx_cc_hook()
    partition_name = nc.partition_id_tensor.name if nc.partition_id_tensor else None
    in_names, out_names, out_avals = [], [], []
    for alloc in nc.m.functions[0].allocations:
        if not isinstance(alloc, mybir.MemoryLocationSet):
            continue
        name = alloc.memorylocations[0].name
        if alloc.kind == "ExternalInput":
            if name != partition_name:
                in_names.append(name)
        elif alloc.kind == "ExternalOutput":
            out_names.append(name)
            out_avals.append(jax.core.ShapedArray(tuple(alloc.tensor_shape),
                                                  mybir.dt.np(alloc.dtype)))
    n_params = len(in_names)
    all_in = list(in_names) + list(out_names)
    if partition_name is not None:
        all_in.append(partition_name)

    def _body(*args):
        operands = list(args)
        if partition_name is not None:
            operands.append(partition_id_tensor())
        return tuple(_bass_exec_p.bind(
            *operands, out_avals=tuple(out_avals), in_names=tuple(all_in),
            out_names=tuple(out_names), lowering_input_output_aliases=(),
            sim_require_finite=True, sim_require_nnan=True, nc=nc))

    devices = jax.devices()[:NCORES]
    assert len(devices) == NCORES
    mesh = Mesh(np.asarray(devices), ("core",))
    specs = (PartitionSpec("core"),) * (n_params + len(out_names))
    fn = jax.jit(shard_map(_body, mesh=mesh, in_specs=specs,
                           out_specs=(PartitionSpec("core"),) * len(out_names),
                           check_rep=False), keep_unused=True)
    return fn, in_names, out_names, out_avals


def _run_cached(nc, in_maps):
    """Run via cached jitted executable with retry; fall back to
    run_bass_kernel_spmd. Retries cover transient device wedges
    (NRT_EXEC_UNIT_UNRECOVERABLE) seen after rapid process turnover."""
    import time as _time
    last_exc = None
    for attempt in range(3):
        try:
            return _run_once(nc, in_maps)
        except Exception as e:
            last_exc = e
            _RUNNER_CACHE.pop(id(nc), None)
            _time.sleep(10 * (attempt + 1))
    raise last_exc


def _run_once(nc, in_maps):
    import jax
    try:
        from concourse._compat import axon_active
        if not axon_active():
            raise RuntimeError("not axon; use native path")
        key = id(nc)
        if key not in _RUNNER_CACHE:
            _RUNNER_CACHE[key] = _pjrt_runner(nc)
        fn, in_names, out_names, out_avals = _RUNNER_CACHE[key]
        concat_in = [np.concatenate([np.asarray(in_maps[c][k])
                                     for c in range(NCORES)], axis=0)
                     for k in in_names]
        concat_zeros = [np.zeros((NCORES * a.shape[0], *a.shape[1:]), a.dtype)
                        for a in out_avals]
        outs = fn(*concat_in, *concat_zeros)
        jax.block_until_ready(outs)
        out_np = [np.asarray(o) for o in outs]
        return [{name: out_np[i].reshape(NCORES, *out_avals[i].shape)[c]
                 for i, name in enumerate(out_names)}
                for c in range(NCORES)]
    except Exception:
        res = run_bass_kernel_spmd(nc, in_maps, core_ids=list(range(NCORES)))
        return res.results


def _get_nc(bl, fast_affine):
    key = (bl, fast_affine)
    if key not in _NC_CACHE:
        cfg = None if fast_affine else {"xload": 2, "cp": 1, "gp": 1, "h32": 1, "bp": 1}
        _NC_CACHE[key] = _build(bl, fast_affine, cfg)
    return _NC_CACHE[key]


def _q8_pair(a):
    """fp8 hi/lo residual pair of a (f32). Residual left unscaled (it fits
    fp8's dynamic range); hi + lo together carry ~bf16 precision."""
    hi = a.astype(NP8)
    lo = (a - hi.astype(np.float32)).astype(NP8)
    return hi, lo


def _w_layout(w):
    """[D_in, D_out] -> [P, NK, D_out] with dim1 = (chunk c, plane i):
    input feature f = c*256 + i*128 + p."""
    d_in, d_out = w.shape
    return np.ascontiguousarray(
        w.reshape(d_in // 256, 2, P, d_out).transpose(2, 0, 1, 3)
        .reshape(P, d_in // P, d_out))


def kernel(z_s, z_e, router_w, router_b, w1, b1, w2, b2, gamma, beta):
    z_s = np.ascontiguousarray(
        np.asarray(z_s, dtype=np.float32).astype(ml_dtypes.bfloat16))
    z_e = np.ascontiguousarray(
        np.asarray(z_e, dtype=np.float32).astype(ml_dtypes.bfloat16))
    router_w = np.asarray(router_w, dtype=np.float32)
    router_b = np.asarray(router_b, dtype=np.float32)
    w1 = np.asarray(w1, dtype=np.float32)
    b1 = np.asarray(b1, dtype=np.float32)
    w2 = np.asarray(w2, dtype=np.float32)
    b2 = np.asarray(b2, dtype=np.float32)
    gamma = np.ascontiguousarray(np.asarray(gamma, dtype=np.float32))
    beta = np.ascontiguousarray(np.asarray(beta, dtype=np.float32))

    b_full = z_s.shape[0]
    assert b_full % NCORES == 0, f"batch {b_full} not divisible by {NCORES} cores"
    bl = b_full // NCORES
    assert bl % 512 == 0, f"per-core batch {bl} must be a multiple of 512"

    fast_affine = bool(np.all(gamma == 1.0) and np.all(beta == 0.0))
    nc = _get_nc(bl, fast_affine)

    # static parameter preprocessing: fp8 residual pairs in PE layout
    rwh, rwl = _q8_pair(WS * router_w)
    rwh = _w_layout(rwh.astype(np.float32)).astype(NP8)
    rwl = _w_layout(rwl.astype(np.float32)).astype(NP8)
    w1h_l, w1l_l, w2h_l, w2l_l = [], [], [], []
    for e in range(E):
        h, l = _q8_pair(WS * w1[e])
        w1h_l.append(_w_layout(h.astype(np.float32)))
        w1l_l.append(_w_layout(l.astype(np.float32)))
        h, l = _q8_pair(WS * w2[e])
        w2h_l.append(_w_layout(h.astype(np.float32)))
        w2l_l.append(_w_layout(l.astype(np.float32)))
    w1h = np.stack(w1h_l).astype(NP8)
    w1l = np.stack(w1l_l).astype(NP8)
    w2h = np.stack(w2h_l).astype(NP8)
    w2l = np.stack(w2l_l).astype(NP8)
    b1_r = np.ascontiguousarray(b1.reshape(E, NK, P))
    b2h, b2l = _q8_pair(WS * b2)
    b2p = np.ascontiguousarray(
        np.stack([b2h, b2l], axis=1).reshape(E, 1, 2, D))
    rb_r = np.ascontiguousarray((WS * router_b).reshape(1, E))

    ident = np.eye(P, dtype=np.float32).astype(ml_dtypes.bfloat16)
    ones_h = np.ones((1, P), dtype=np.float32)
    ones8 = np.ones((1, 2, P), dtype=np.float32).astype(NP8)
    in_maps = []
    for c in range(NCORES):
        sl = slice(c * bl, (c + 1) * bl)
        in_maps.append({
            "zs": z_s[sl], "ze": z_e[sl],
            "rwh": rwh, "rwl": rwl, "rb": rb_r,
            "w1h": w1h, "w1l": w1l, "w2h": w2h, "w2l": w2l,
            "b1": b1_r, "b2p": b2p,
            "gam": gamma, "bet": beta,
            "ident": ident, "ones": ones_h, "ones8": ones8,
        })
    results = _run_cached(nc, in_maps)
    return np.concatenate([results[c]["z"] for c in range(NCORES)], axis=0)


# revision 32
# speedup vs baseline: 1.0052x; 1.0020x over previous
"""MoE fusion kernel for Trainium2, data-parallel across 8 NeuronCores.

Reference computation (per row b of B=16384):
    x      = concat(z_s, z_e)                    # [1024]
    wgt    = softmax(x @ rw + rb)                # [8]
    h_e    = gelu(x @ w1[e] + b1[e])             # [8, 1024]
    y_e    = h_e @ w2[e] + b2[e]                 # [8, 1024]
    ln_e   = (y_e - mu_e) * rsqrt(var_e + eps) * gamma[e] + beta[e]
    z      = sum_e wgt[e] * ln_e                 # [1024]

Sharding: batch split 8 ways (2048 rows/core), params replicated. No
collectives.

GEMMs run in fp8(e4m3) with the tensor engine's DoubleRow perf mode
(contracts 2x128 per matmul at 0.5 cycles/row = 4x the fp32r MAC rate).
To stay within the error budget each GEMM uses three DoubleRow passes
(residual "virtual bf16" quantization):

    a @ w  ~=  a_hi @ w_hi  +  a_lo @ w_hi  +  a_hi @ w_lo

where a_hi = fp8(a), a_lo = fp8(a - a_hi) (residuals stay unscaled:
they fit fp8's dynamic range), and weights are pre-scaled by 128 on the
host so their values sit in fp8's normal range. All passes accumulate
into one fp32 PSUM group whose overall scale is 128; the gelu drain
rescales by 1/128 (activation scale), and the layer-2 LayerNorm is
computed directly on the x128 PSUM (LN is scale-invariant; eps is
scaled by 128^2). b2 rides the layer-2 PSUM group as a K=1 DoubleRow
ones-row matmul of its own fp8 hi/lo pair; b1 rides the gelu
activation's per-partition bias.

Weight residual pairs are quantized on the host (static parameter
preprocessing); activation hi/lo tensors are produced on device (act
casts x_hi from transpose PSUM; DVE casts h_hi and computes the lo
residuals with fused scalar_tensor_tensor). The softmax router runs
the same fp8 hi/lo scheme (logits scale 1/128 folded into the Exp
activation). LayerNorm's rstd is a 3-step Newton rsqrt on DVE
(constant seed) so the act engine never leaves the gelu table set
(act-function table reloads cost ~5us each); the weighted expert
accumulation of normalized outputs runs on the gpsimd (Pool) engine.
The schedule software-pipelines layer 2 one half-tile behind layer 1
through a shared 8-bank PSUM ring.
"""
import numpy as np
import ml_dtypes
from contextlib import ExitStack

import concourse.bass as bass
import concourse.bacc as bacc
import concourse.mybir as mybir
import concourse.tile as tile
from concourse.bass_utils import run_bass_kernel_spmd

P = 128          # partitions
D = 1024         # IN_DIM == OUT_DIM
E = 8            # experts
NK = D // P      # 8 contraction chunks of 128
NC2 = NK // 2    # 4 DoubleRow chunks of 256
NCORES = 8
B_FULL = 16384
BL = B_FULL // NCORES   # 2048 rows per core
SEQ = 512               # z_s/z_e width
WS = 128.0              # host weight pre-scale (power of two)

F32 = mybir.dt.float32
F32R = mybir.dt.float32r
F8 = mybir.dt.float8e4
BF16 = mybir.dt.bfloat16
NP8 = ml_dtypes.float8_e4m3
AF = mybir.ActivationFunctionType
ALU = mybir.AluOpType
DR = mybir.MatmulPerfMode.DoubleRow


def _build(bl, fast_affine, cfg=None):
    """Build the per-core Bass program. bl: rows per core."""
    cfg = cfg or {}
    nb = bl // P            # 128-row chunks
    nt = bl // 512          # 512-row tiles

    nc = bacc.Bacc(None, target_bir_lowering=False)
    zs_d = nc.declare_dram_parameter("zs", [bl, SEQ], BF16, isOutput=False)
    ze_d = nc.declare_dram_parameter("ze", [bl, SEQ], BF16, isOutput=False)
    rwh_d = nc.declare_dram_parameter("rwh", [P, NK, E], F8, isOutput=False)
    rwl_d = nc.declare_dram_parameter("rwl", [P, NK, E], F8, isOutput=False)
    rb_d = nc.declare_dram_parameter("rb", [1, E], F32R, isOutput=False)
    w1h_d = nc.declare_dram_parameter("w1h", [E, P, NK, D], F8, isOutput=False)
    w1l_d = nc.declare_dram_parameter("w1l", [E, P, NK, D], F8, isOutput=False)
    w2h_d = nc.declare_dram_parameter("w2h", [E, P, NK, D], F8, isOutput=False)
    w2l_d = nc.declare_dram_parameter("w2l", [E, P, NK, D], F8, isOutput=False)
    b1_d = nc.declare_dram_parameter("b1", [E, NK, P], F32, isOutput=False)
    b2p_d = nc.declare_dram_parameter("b2p", [E, 1, 2, D], F8, isOutput=False)
    gam_d = nc.declare_dram_parameter("gam", [E, D], F32, isOutput=False)
    bet_d = nc.declare_dram_parameter("bet", [E, D], F32, isOutput=False)
    id_d = nc.declare_dram_parameter("ident", [P, P], BF16, isOutput=False)
    on_d = nc.declare_dram_parameter("ones", [1, P], F32R, isOutput=False)
    on8_d = nc.declare_dram_parameter("ones8", [1, 2, P], F8, isOutput=False)
    z_d = nc.declare_dram_parameter("z", [bl, D], F32, isOutput=True)

    with tile.TileContext(nc) as tc, ExitStack() as ctx:
        consts = ctx.enter_context(tc.tile_pool(name="consts", bufs=1))
        xload = ctx.enter_context(tc.tile_pool(name="xload", bufs=cfg.get("xload", 6)))
        xtp = ctx.enter_context(tc.tile_pool(name="xtp", bufs=1))
        wp = ctx.enter_context(tc.tile_pool(name="wp", bufs=cfg.get("w", 2)))
        bp = ctx.enter_context(tc.tile_pool(name="bp", bufs=cfg.get("bp", 2)))
        hp = ctx.enter_context(tc.tile_pool(name="hp", bufs=cfg.get("h", 2)))
        h32p = ctx.enter_context(tc.tile_pool(name="h32p", bufs=cfg.get("h32", 3)))
        zp = ctx.enter_context(tc.tile_pool(name="zp", bufs=nb))
        cp = ctx.enter_context(tc.tile_pool(name="cp", bufs=cfg.get("cp", 3)))
        sp = ctx.enter_context(tc.tile_pool(name="sp", bufs=8))
        gp = None
        if not fast_affine:
            gp = ctx.enter_context(tc.tile_pool(name="gp", bufs=cfg.get("gp", 1)))
        psA = ctx.enter_context(tc.tile_pool(name="psA", bufs=cfg.get("psA", 8), space="PSUM"))
        psB = psA

        ident = consts.tile([P, P], BF16)
        nc.sync.dma_start(out=ident, in_=id_d[:])
        seed_t = consts.tile([P, 2], F32)
        nc.vector.memset(seed_t, 0.030)
        ones_t = consts.tile([1, P], F32R)
        nc.sync.dma_start(out=ones_t, in_=on_d[:])
        ones8_t = consts.tile([1, 2, P], F8)
        nc.sync.dma_start(out=ones8_t, in_=on8_d[:])
        rwh_sb = consts.tile([P, NK, E], F8)
        nc.sync.dma_start(out=rwh_sb, in_=rwh_d[:])
        rwl_sb = consts.tile([P, NK, E], F8)
        nc.sync.dma_start(out=rwl_sb, in_=rwl_d[:])
        rb_sb = consts.tile([1, E], F32R)
        nc.sync.dma_start(out=rb_sb, in_=rb_d[:])


        def load_expert(e):
            w1h = wp.tile([P, NK, D], F8, tag="w1h", name=f"w1h_{e}")
            nc.sync.dma_start(out=w1h, in_=w1h_d[e])
            w1l = wp.tile([P, NK, D], F8, tag="w1l", name=f"w1l_{e}")
            nc.sync.dma_start(out=w1l, in_=w1l_d[e])
            w2h = wp.tile([P, NK, D], F8, tag="w2h", name=f"w2h_{e}")
            nc.sync.dma_start(out=w2h, in_=w2h_d[e])
            w2l = wp.tile([P, NK, D], F8, tag="w2l", name=f"w2l_{e}")
            nc.sync.dma_start(out=w2l, in_=w2l_d[e])
            b1_sb = bp.tile([P, NK], F32, tag="b1", name=f"b1_{e}")
            nc.sync.dma_start(out=b1_sb, in_=b1_d[e].rearrange("m p -> p m"))
            b2p_sb = bp.tile([1, 2, D], F8, tag="b2", name=f"b2_{e}")
            nc.sync.dma_start(out=b2p_sb, in_=b2p_d[e])
            gam_sb = bet_sb = None
            if not fast_affine:
                gam_sb = gp.tile([P, D], F32, tag="g", name=f"g_{e}")
                nc.sync.dma_start(out=gam_sb, in_=gam_d[e].partition_broadcast(P))
                bet_sb = gp.tile([P, D], F32, tag="bt", name=f"bt_{e}")
                nc.sync.dma_start(out=bet_sb, in_=bet_d[e].partition_broadcast(P))
            return dict(w1h=w1h, w1l=w1l, w2h=w2h, w2l=w2l, b1=b1_sb,
                        b2p=b2p_sb, gam=gam_sb, bet=bet_sb)

        def l1_half(e, t, w, hh, hl, m0, m1):
            """Layer 1: hT chunks [feat 128, batch 512], 3-pass fp8.
            The fp8 hi/lo casts are deferred (returned) so the L2 pair's
            LN chain gets DVE queue priority."""
            deferred = []
            ts = slice(t * 512, (t + 1) * 512)
            w1h, w1l, b1_sb = w["w1h"], w["w1l"], w["b1"]
            for m in range(m0, m1):
                ps_h = psA.tile([P, 512], F32, tag="a", name=f"ph_{e}_{t}_{m}")
                ms = slice(m * P, (m + 1) * P)
                for c in range(NC2):
                    cs = slice(2 * c, 2 * c + 2)
                    nc.tensor.matmul(ps_h, w1h[:, cs, ms], xh[:, cs, ts],
                                     start=(c == 0), stop=False, perf_mode=DR)
                    nc.tensor.matmul(ps_h, w1h[:, cs, ms], xl[:, cs, ts],
                                     start=False, stop=False, perf_mode=DR)
                for c in range(NC2):
                    cs = slice(2 * c, 2 * c + 2)
                    nc.tensor.matmul(ps_h, w1l[:, cs, ms], xh[:, cs, ts],
                                     start=False, stop=(c == NC2 - 1),
                                     perf_mode=DR)
                h32 = h32p.tile([P, 512], BF16, tag="h32", name=f"h32_{e}_{t}_{m}")
                nc.scalar.activation(out=h32, in_=ps_h, func=AF.Gelu,
                                     bias=b1_sb[:, m:m + 1], scale=1.0 / WS)
                deferred.append((h32, m))
            return deferred

        def flush_casts(hh, hl, deferred):
            for h32, m in deferred:
                nc.vector.tensor_scalar(out=hh[:, m, :], in0=h32, scalar1=1.0,
                                        scalar2=None, op0=ALU.mult)
                nc.vector.scalar_tensor_tensor(
                    out=hl[:, m, :], in0=h32, scalar=1.0, in1=hh[:, m, :],
                    op0=ALU.mult, op1=ALU.subtract)

        def l2_pair(e, t, hh, hl, w, sp2):
            """Layer 2 + LN + weighted accumulate, paired row chunks.

            Two 128-row chunks' PSUMs (4 banks) stay live while one packed
            [P, 2] LN-scalar chain (bn_aggr + Newton rsqrt on DVE; no
            act-engine table switch) computes alpha/nbias for both, then
            the act engine applies and Pool accumulates z."""
            w2h, w2l, b2p_sb = w["w2h"], w["w2l"], w["b2p"]
            gam_sb, bet_sb = w["gam"], w["bet"]
            if True:
                ps_pair = []
                stats = sp.tile([P, 2, 2, 6], F32, tag="st",
                                name=f"st_{e}_{t}_{sp2}")
                for sh in range(2):
                    s = sp2 * 2 + sh
                    bb = t * 4 + s
                    ss = slice(s * P, (s + 1) * P)
                    ps_ys = [psB.tile([P, 512], F32, tag="a",
                                      name=f"py_{e}_{bb}_{n}")
                             for n in range(2)]
                    # c-outer: consecutive matmuls share the stationary
                    # h chunk
                    for c in range(NC2):
                        cs = slice(2 * c, 2 * c + 2)
                        for n in range(2):
                            ns = slice(n * 512, (n + 1) * 512)
                            nc.tensor.matmul(ps_ys[n], hh[:, cs, ss],
                                             w2h[:, cs, ns], start=(c == 0),
                                             stop=False, perf_mode=DR)
                            nc.tensor.matmul(ps_ys[n], hh[:, cs, ss],
                                             w2l[:, cs, ns], start=False,
                                             stop=False, perf_mode=DR)
                    for c in range(NC2):
                        cs = slice(2 * c, 2 * c + 2)
                        for n in range(2):
                            nc.tensor.matmul(ps_ys[n], hl[:, cs, ss],
                                             w2h[:, cs, n * 512:(n + 1) * 512],
                                             start=False, stop=False,
                                             perf_mode=DR)
                    for n in range(2):
                        nc.tensor.matmul(ps_ys[n], ones8_t,
                                         b2p_sb[:, :, n * 512:(n + 1) * 512],
                                         start=False, stop=True, perf_mode=DR)
                        nc.vector.bn_stats(out=stats[:, sh, n, :],
                                           in_=ps_ys[n])
                    ps_pair.append(ps_ys)
                mv = sp.tile([P, 2, 2], F32, tag="mv", name=f"mv_{e}_{t}_{sp2}")
                for sh in range(2):
                    nc.vector.bn_aggr(out=mv[:, sh, :], in_=stats[:, sh, :, :])
                vpe = sp.tile([P, 2], F32, tag="vp", name=f"vp_{e}_{t}_{sp2}")
                nc.vector.tensor_scalar(out=vpe, in0=mv[:, :, 1],
                                        scalar1=1e-5 * WS * WS, scalar2=None,
                                        op0=ALU.add)
                # rstd = rsqrt(vpe), Newton from constant seed (typical row
                # std of the x128 psum is ~33 -> rstd ~0.03)
                r = seed_t
                for it in range(3):
                    sq = sp.tile([P, 2], F32, tag=f"sq{it}",
                                 name=f"sq_{e}_{t}_{sp2}_{it}")
                    nc.vector.tensor_mul(sq, r, r)
                    nc.vector.tensor_mul(sq, sq, vpe)
                    nc.vector.tensor_scalar(out=sq, in0=sq, scalar1=-0.5,
                                            scalar2=1.5, op0=ALU.mult,
                                            op1=ALU.add)
                    r2 = sp.tile([P, 2], F32, tag=f"r{it}",
                                 name=f"r_{e}_{t}_{sp2}_{it}")
                    nc.vector.tensor_mul(r2, r, sq)
                    r = r2
                alpha = sp.tile([P, 2], F32, tag="al", name=f"al_{e}_{t}_{sp2}")
                nc.vector.tensor_mul(alpha, r,
                                     wsm[:, t * 4 + sp2 * 2:t * 4 + sp2 * 2 + 2, e])
                nbias = sp.tile([P, 2], F32, tag="nb", name=f"nb_{e}_{t}_{sp2}")
                nc.vector.scalar_tensor_tensor(out=nbias, in0=mv[:, :, 0],
                                               scalar=-1.0, in1=alpha,
                                               op0=ALU.mult, op1=ALU.mult)
                for sh in range(2):
                    s = sp2 * 2 + sh
                    bb = t * 4 + s
                    ps_ys = ps_pair[sh]
                    al_s = alpha[:, sh:sh + 1]
                    nb_s = nbias[:, sh:sh + 1]
                    for n in range(2):
                        ns = slice(n * 512, (n + 1) * 512)
                        zslice = z_t[bb][:, ns]
                        if fast_affine and e == 0:
                            nc.scalar.activation(out=zslice, in_=ps_ys[n],
                                                 func=AF.Identity,
                                                 bias=nb_s, scale=al_s)
                        else:
                            ct = cp.tile([P, 512], F32, tag="ct",
                                         name=f"ct_{e}_{bb}_{n}")
                            nc.scalar.activation(out=ct, in_=ps_ys[n],
                                                 func=AF.Identity,
                                                 bias=nb_s, scale=al_s)
                            if not fast_affine:
                                nc.vector.tensor_mul(ct, ct, gam_sb[:, ns])
                                bw = cp.tile([P, 512], F32, tag="bw",
                                             name=f"bw_{e}_{bb}_{n}")
                                nc.vector.tensor_scalar_mul(
                                    out=bw, in0=bet_sb[:, ns],
                                    scalar1=wsm[:, bb, e:e + 1])
                                if e == 0:
                                    nc.gpsimd.tensor_add(zslice, ct, bw)
                                else:
                                    nc.gpsimd.tensor_add(zslice, zslice, bw)
                                    nc.gpsimd.tensor_add(zslice, zslice, ct)
                            else:
                                nc.gpsimd.tensor_add(zslice, zslice, ct)
                        if e == E - 1 and n == 1:
                            nc.sync.dma_start(
                                out=z_d[bb * P:(bb + 1) * P, :], in_=z_t[bb])



        # ---- transpose x into feature-major fp8 hi/lo + router weights ----
        xh = xtp.tile([P, NK, bl], F8, name="xh")
        xl = xtp.tile([P, NK, bl], F8, name="xl")
        wsm = xtp.tile([P, nb, E], F32, name="wsm")
        for b in range(nb):
            x_sb = xload.tile([P, D], BF16, tag="x", name=f"x_{b}")
            row = b * P
            nc.sync.dma_start(out=x_sb[:, :SEQ], in_=zs_d[row:row + P, :])
            nc.sync.dma_start(out=x_sb[:, SEQ:], in_=ze_d[row:row + P, :])
            for q in range(2):
                tp = psA.tile([P, 4, P], BF16, tag="a", name=f"tp_{b}_{q}")
                for j in range(4):
                    c = q * 4 + j
                    nc.tensor.transpose(tp[:, j, :], x_sb[:, c * P:(c + 1) * P],
                                        ident)
                nc.scalar.activation(
                    out=xh[:, q * 4:(q + 1) * 4, b * P:(b + 1) * P], in_=tp,
                    func=AF.Copy)
                nc.vector.scalar_tensor_tensor(
                    out=xl[:, q * 4:(q + 1) * 4, b * P:(b + 1) * P], in0=tp,
                    scalar=1.0, in1=xh[:, q * 4:(q + 1) * 4, b * P:(b + 1) * P],
                    op0=ALU.mult, op1=ALU.subtract)
            # router logits for this 128-row chunk (fp8 hi/lo + f32r bias row)
            ps_r = psA.tile([P, E], F32, tag="a", name=f"psr_{b}")
            for c in range(NC2):
                nc.tensor.matmul(ps_r, xh[:, 2 * c:2 * c + 2, b * P:(b + 1) * P],
                                 rwh_sb[:, 2 * c:2 * c + 2, :],
                                 start=(c == 0), stop=False, perf_mode=DR)
            for c in range(NC2):
                nc.tensor.matmul(ps_r, xh[:, 2 * c:2 * c + 2, b * P:(b + 1) * P],
                                 rwl_sb[:, 2 * c:2 * c + 2, :],
                                 start=False, stop=False, perf_mode=DR)
                nc.tensor.matmul(ps_r, xl[:, 2 * c:2 * c + 2, b * P:(b + 1) * P],
                                 rwh_sb[:, 2 * c:2 * c + 2, :],
                                 start=False, stop=False, perf_mode=DR)
            nc.tensor.matmul(ps_r, ones_t, rb_sb, start=False, stop=True)
            ex = sp.tile([P, E], F32, tag="ex", name=f"ex_{b}")
            nc.scalar.activation(out=ex, in_=ps_r, func=AF.Exp, scale=1.0 / WS)
            sm = sp.tile([P, 1], F32, tag="sm", name=f"sm_{b}")
            nc.vector.tensor_reduce(out=sm, in_=ex, axis=mybir.AxisListType.X,
                                    op=ALU.add)
            rc = sp.tile([P, 1], F32, tag="rc", name=f"rc_{b}")
            nc.vector.reciprocal(out=rc, in_=sm)
            nc.vector.tensor_scalar_mul(out=wsm[:, b, :], in0=ex, scalar1=rc)

        z_t = [zp.tile([P, D], F32, tag="z", name=f"z_{b}") for b in range(nb)]

        # ---- pipelined expert loop: L2 lags L1 by one 512-row tile ----
        # (expert 0's weights were DMA'd before the x preamble so they are
        # resident when its first L1 matmuls issue)
        pend = None
        wcur = None
        for k in range(E * nt + 1):
            if k < E * nt:
                e, t = divmod(k, nt)
                if t == 0:
                    wcur = load_expert(e)
                hh = hp.tile([P, NK, 512], F8, tag="hh", name=f"hh_{e}_{t}")
                hl = hp.tile([P, NK, 512], F8, tag="hl", name=f"hl_{e}_{t}")
                d0 = l1_half(e, t, wcur, hh, hl, 0, NK // 2)
                if pend is not None:
                    l2_pair(pend[0], pend[1], pend[2], pend[3], pend[4], 0)
                flush_casts(hh, hl, d0)
                d1 = l1_half(e, t, wcur, hh, hl, NK // 2, NK)
                if pend is not None:
                    l2_pair(pend[0], pend[1], pend[2], pend[3], pend[4], 1)
                flush_casts(hh, hl, d1)
                pend = (e, t, hh, hl, wcur)
            else:
                l2_pair(pend[0], pend[1], pend[2], pend[3], pend[4], 0)
                l2_pair(pend[0], pend[1], pend[2], pend[3], pend[4], 1)

    nc.compile()
    return nc


_NC_CACHE = {}
_RUNNER_CACHE = {}


def _pjrt_runner(nc):
    """Reusable jitted PJRT executable for `nc` (axon path). Mirrors
    bass2jax.run_bass_via_pjrt but is cached so repeated kernel() calls do
    not re-trace/recompile."""
    import jax
    from jax.sharding import Mesh, PartitionSpec
    from jax.experimental.shard_map import shard_map
    from concourse.bass2jax import (_bass_exec_p, install_neuronx_cc_hook,
                                    partition_id_tensor)

    install_neuronx_cc_hook()
    partition_name = nc.partition_id_tensor.name if nc.partition_id_tensor else None
    in_names, out_names, out_avals = [], [], []
    for alloc in nc.m.functions[0].allocations:
        if not isinstance(alloc, mybir.MemoryLocationSet):
            continue
        name = alloc.memorylocations[0].name
        if alloc.kind == "ExternalInput":
            if name != partition_name:
                in_names.append(name)
        elif alloc.kind == "ExternalOutput":
            out_names.append(name)
            out_avals.append(jax.core.ShapedArray(tuple(alloc.tensor_shape),
                                                  mybir.dt.np(alloc.dtype)))
    n_params = len(in_names)
    all_in = list(in_names) + list(out_names)
    if partition_name is not None:
        all_in.append(partition_name)

    def _body(*args):
        operands = list(args)
        if partition_name is not None:
            operands.append(partition_id_tensor())
        return tuple(_bass_exec_p.bind(
            *operands, out_avals=tuple(out_avals), in_names=tuple(all_in),
            out_names=tuple(out_names), lowering_input_output_aliases=(),
            sim_require_finite=True, sim_require_nnan=True, nc=nc))

    devices = jax.devices()[:NCORES]
    assert len(devices) == NCORES
    mesh = Mesh(np.asarray(devices), ("core",))
    specs = (PartitionSpec("core"),) * (n_params + len(out_names))
    fn = jax.jit(shard_map(_body, mesh=mesh, in_specs=specs,
                           out_specs=(PartitionSpec("core"),) * len(out_names),
                           check_rep=False), keep_unused=True)
    return fn, in_names, out_names, out_avals


def _run_cached(nc, in_maps):
    """Run via cached jitted executable with retry; fall back to
    run_bass_kernel_spmd. Retries cover transient device wedges
    (NRT_EXEC_UNIT_UNRECOVERABLE) seen after rapid process turnover."""
    import time as _time
    last_exc = None
    for attempt in range(3):
        try:
            return _run_once(nc, in_maps)
        except Exception as e:
            last_exc = e
            _RUNNER_CACHE.pop(id(nc), None)
            _time.sleep(10 * (attempt + 1))
    raise last_exc


def _run_once(nc, in_maps):
    import jax
    try:
        from concourse._compat import axon_active
        if not axon_active():
            raise RuntimeError("not axon; use native path")
        key = id(nc)
        if key not in _RUNNER_CACHE:
            _RUNNER_CACHE[key] = _pjrt_runner(nc)
        fn, in_names, out_names, out_avals = _RUNNER_CACHE[key]
        concat_in = [np.concatenate([np.asarray(in_maps[c][k])
                                     for c in range(NCORES)], axis=0)
                     for k in in_names]
        concat_zeros = [np.zeros((NCORES * a.shape[0], *a.shape[1:]), a.dtype)
                        for a in out_avals]
        outs = fn(*concat_in, *concat_zeros)
        jax.block_until_ready(outs)
        out_np = [np.asarray(o) for o in outs]
        return [{name: out_np[i].reshape(NCORES, *out_avals[i].shape)[c]
                 for i, name in enumerate(out_names)}
                for c in range(NCORES)]
    except Exception:
        res = run_bass_kernel_spmd(nc, in_maps, core_ids=list(range(NCORES)))
        return res.results


def _get_nc(bl, fast_affine):
    key = (bl, fast_affine)
    if key not in _NC_CACHE:
        cfg = None if fast_affine else {"xload": 2, "cp": 1, "gp": 1, "h32": 1, "bp": 1}
        _NC_CACHE[key] = _build(bl, fast_affine, cfg)
    return _NC_CACHE[key]


def _q8_pair(a):
    """fp8 hi/lo residual pair of a (f32). Residual left unscaled (it fits
    fp8's dynamic range); hi + lo together carry ~bf16 precision."""
    hi = a.astype(NP8)
    lo = (a - hi.astype(np.float32)).astype(NP8)
    return hi, lo


def _w_layout(w):
    """[D_in, D_out] -> [P, NK, D_out] with dim1 = (chunk c, plane i):
    input feature f = c*256 + i*128 + p."""
    d_in, d_out = w.shape
    return np.ascontiguousarray(
        w.reshape(d_in // 256, 2, P, d_out).transpose(2, 0, 1, 3)
        .reshape(P, d_in // P, d_out))


def kernel(z_s, z_e, router_w, router_b, w1, b1, w2, b2, gamma, beta):
    z_s = np.ascontiguousarray(
        np.asarray(z_s, dtype=np.float32).astype(ml_dtypes.bfloat16))
    z_e = np.ascontiguousarray(
        np.asarray(z_e, dtype=np.float32).astype(ml_dtypes.bfloat16))
    router_w = np.asarray(router_w, dtype=np.float32)
    router_b = np.asarray(router_b, dtype=np.float32)
    w1 = np.asarray(w1, dtype=np.float32)
    b1 = np.asarray(b1, dtype=np.float32)
    w2 = np.asarray(w2, dtype=np.float32)
    b2 = np.asarray(b2, dtype=np.float32)
    gamma = np.ascontiguousarray(np.asarray(gamma, dtype=np.float32))
    beta = np.ascontiguousarray(np.asarray(beta, dtype=np.float32))

    b_full = z_s.shape[0]
    assert b_full % NCORES == 0, f"batch {b_full} not divisible by {NCORES} cores"
    bl = b_full // NCORES
    assert bl % 512 == 0, f"per-core batch {bl} must be a multiple of 512"

    fast_affine = bool(np.all(gamma == 1.0) and np.all(beta == 0.0))
    nc = _get_nc(bl, fast_affine)

    # static parameter preprocessing: fp8 residual pairs in PE layout
    rwh, rwl = _q8_pair(WS * router_w)
    rwh = _w_layout(rwh.astype(np.float32)).astype(NP8)
    rwl = _w_layout(rwl.astype(np.float32)).astype(NP8)
    w1h_l, w1l_l, w2h_l, w2l_l = [], [], [], []
    for e in range(E):
        h, l = _q8_pair(WS * w1[e])
        w1h_l.append(_w_layout(h.astype(np.float32)))
        w1l_l.append(_w_layout(l.astype(np.float32)))
        h, l = _q8_pair(WS * w2[e])
        w2h_l.append(_w_layout(h.astype(np.float32)))
        w2l_l.append(_w_layout(l.astype(np.float32)))
    w1h = np.stack(w1h_l).astype(NP8)
    w1l = np.stack(w1l_l).astype(NP8)
    w2h = np.stack(w2h_l).astype(NP8)
    w2l = np.stack(w2l_l).astype(NP8)
    b1_r = np.ascontiguousarray(b1.reshape(E, NK, P))
    b2h, b2l = _q8_pair(WS * b2)
    b2p = np.ascontiguousarray(
        np.stack([b2h, b2l], axis=1).reshape(E, 1, 2, D))
    rb_r = np.ascontiguousarray((WS * router_b).reshape(1, E))

    ident = np.eye(P, dtype=np.float32).astype(ml_dtypes.bfloat16)
    ones_h = np.ones((1, P), dtype=np.float32)
    ones8 = np.ones((1, 2, P), dtype=np.float32).astype(NP8)
    in_maps = []
    for c in range(NCORES):
        sl = slice(c * bl, (c + 1) * bl)
        in_maps.append({
            "zs": z_s[sl], "ze": z_e[sl],
            "rwh": rwh, "rwl": rwl, "rb": rb_r,
            "w1h": w1h, "w1l": w1l, "w2h": w2h, "w2l": w2l,
            "b1": b1_r, "b2p": b2p,
            "gam": gamma, "bet": beta,
            "ident": ident, "ones": ones_h, "ones8": ones8,
        })
    results = _run_cached(nc, in_maps)
    return np.concatenate([results[c]["z"] for c in range(NCORES)], axis=0)
